# revision 8
# baseline (speedup 1.0000x reference)
"""CoverageLoss kernel for 8 Trainium2 NeuronCores.

Redesign vs the thermometer-quantized-L1 kernel (150us):
  Candidate metric is squared-L2 via one bf16 K=e+1 matmul per 512-latent
  chunk (score = <s,l> - |l|^2/2, monotone in -L2^2 per row) instead of a
  Q=16 thermometer (K=1024) -- 16x less contraction and bf16-rate
  streaming (fp32 moving operands measured ~5x slower).  Per-row top-k
  runs on a group-max-coarsened score row: reduce groups of G=32 latents
  to their max, then MAX8/FIND_INDEX8 scan only [128, 64] group maxima
  (MAX8/FI8 have no fast DVE perf mode, so shrinking their input 32x is
  the only lever).  Top-8-of-group-maxima covers the true top-8 elements
  (element #k's group ranks <= k).  The group reduce is split across
  engines per PATHS: 'R' tiles use DVE tensor_reduce straight from PSUM
  (1x, but drains PSUM without ScalarE); 'V' tiles use a ScalarE
  psum->bf16 convert plus a DVE halving cascade of tensor_tensor(max)
  (2x_1p) -- groups come out as stride-NGRP lattices {g + NGRP*j} either
  way.  Host expands the winning groups and refines exact L1 there, so
  the device ships only [MS, 8] values+indices per (problem, shard).

Sharding: 4 latent-shards x 2 sample-shards, latents carry a -|l|^2/2 row;
samples carry a ones row.  Host: group expansion, exact L1 refine of global
top-TOP_T groups, tails, top-64 far selection, size losses, final scalar.
"""

from contextlib import ExitStack

import numpy as np

import concourse.bacc as bacc
import concourse.mybir as mybir
import concourse.tile as tile
from concourse.bass_utils import run_bass_kernel_spmd

NLAT, ES, EA = 8192, 64, 32
NSMP = 2048
A_SHARDS, B_SHARDS = 4, 2
NL = NLAT // A_SHARDS              # 2048 latents per core
MS = NSMP // B_SHARDS              # 1024 samples per core
NTILES = MS // 128                 # 8 sample tiles per problem
NCHUNK = NL // 512                 # 4 psum column chunks
G = 32                             # latents per group (stride-NGRP lattice)
NGRP = NL // G                     # 64 groups per shard
TOP_T = 10                         # global top groups refined on host

F32 = mybir.dt.float32
BF16 = mybir.dt.bfloat16
U16 = mybir.dt.uint16

# Per-tile engine routing for the group-max reduce (16 tiles, s/a
# interleaved).  'R': DVE tensor_reduce straight from PSUM (1x but drains
# PSUM without ScalarE).  'V': ScalarE psum->bf16 convert + DVE 3-fold
# tensor_tensor(max) cascade (2x_1p).  'P': ScalarE convert + GPSIMD folds.
# Groups are stride-NGRP lattices {g + NGRP*j} for every path.
PATHS = ['V', 'V', 'V', 'V', 'V', 'R', 'V', 'V',
         'V', 'V', 'V', 'R', 'V', 'V', 'V', 'V']


def _build_nc():
    nc = bacc.Bacc("TRN2", target_bir_lowering=False, debug=False,
                   num_devices=8)
    inp = {}
    for name, shape in [
        ("latK_s", [ES + 1, NL]), ("latK_a", [EA + 1, NL]),
        ("smpK_s", [ES + 1, MS]), ("smpK_a", [EA + 1, MS]),
    ]:
        inp[name] = nc.dram_tensor(name, shape, BF16,
                                   kind="ExternalInput").ap()
    out = {}
    for name, shape, dt in [
        ("gv_s", [MS, 8], F32), ("gi_s", [MS, 8], U16),
        ("gv_a", [MS, 8], F32), ("gi_a", [MS, 8], U16),
    ]:
        out[name] = nc.dram_tensor(name, shape, dt, kind="ExternalOutput").ap()

    with tile.TileContext(nc) as tc, ExitStack() as ctx:
        ins = ctx.enter_context(tc.tile_pool(name="ins", bufs=1))
        sb = ctx.enter_context(tc.tile_pool(name="sb", bufs=4))
        fold = ctx.enter_context(tc.tile_pool(name="fold", bufs=6))
        red = ctx.enter_context(tc.tile_pool(name="red", bufs=6))
        out8 = ctx.enter_context(tc.tile_pool(name="out8", bufs=1))
        ps_pool = {
            "s": ctx.enter_context(tc.tile_pool(name="ps_s", bufs=1,
                                                space="PSUM")),
            "a": ctx.enter_context(tc.tile_pool(name="ps_a", bufs=1,
                                                space="PSUM")),
        }
        prob = {}
        for tag, e in (("s", ES), ("a", EA)):
            K = e + 1
            latT = ins.tile([K, NL], BF16, tag=f"latT_{tag}")
            tc.nc.sync.dma_start(latT[:], inp[f"latK_{tag}"][:, :])
            smpT = ins.tile([K, MS], BF16, tag=f"smpT_{tag}")
            tc.nc.sync.dma_start(smpT[:], inp[f"smpK_{tag}"][:, :])
            v8all = out8.tile([128, NTILES * 8], F32, tag=f"v8all_{tag}")
            i8all = out8.tile([128, NTILES * 8], U16, tag=f"i8all_{tag}")
            prob[tag] = (latT, smpT, v8all, i8all)

        gidx = 0
        for m in range(NTILES):
            for tag in ("s", "a"):
                latT, smpT, v8all, i8all = prob[tag]
                nc_ = tc.nc
                ps = ps_pool[tag].tile([128, NL], F32, tag=f"ps_{tag}")
                for n in range(NCHUNK):
                    nc_.tensor.matmul(
                        ps[:, n * 512:(n + 1) * 512],
                        lhsT=smpT[:, m * 128:(m + 1) * 128],
                        rhs=latT[:, n * 512:(n + 1) * 512],
                        start=True, stop=True)

                rg = red.tile([128, NGRP], BF16, tag=f"rg_{tag}")
                path = PATHS[gidx]
                if path == 'R':
                    nc_.vector.tensor_reduce(
                        out=rg[:],
                        in_=ps[:].rearrange("p (k g) -> p g k", k=G),
                        axis=mybir.AxisListType.X, op=mybir.AluOpType.max)
                else:
                    sbt = sb.tile([128, NL], BF16, tag=f"sb_{tag}")
                    nc_.scalar.copy(sbt[:], ps[:])
                    cur = sbt
                    w = NL
                    while w > 2 * NGRP:
                        w //= 2
                        nxt = fold.tile([128, w], BF16, tag=f"f{w}_{tag}")
                        nc_.vector.tensor_tensor(
                            out=nxt[:], in0=cur[:, :w], in1=cur[:, w:2 * w],
                            op=mybir.AluOpType.max)
                        cur = nxt
                    nc_.vector.tensor_tensor(
                        out=rg[:], in0=cur[:, :NGRP], in1=cur[:, NGRP:],
                        op=mybir.AluOpType.max)

                v8 = v8all[:, m * 8:(m + 1) * 8]
                i8 = i8all[:, m * 8:(m + 1) * 8]
                nc_.vector.max(out=v8, in_=rg[:])
                nc_.vector.max_index(out=i8, in_max=v8, in_values=rg[:])
                gidx += 1
        # one batched output DMA per tensor: SBUF [p, m*8+j] -> DRAM
        # [m*128+p, j]
        for tag in ("s", "a"):
            _, _, v8all, i8all = prob[tag]
            tc.nc.sync.dma_start(
                out[f"gv_{tag}"].rearrange("(m p) j -> p m j", p=128),
                v8all[:])
            tc.nc.sync.dma_start(
                out[f"gi_{tag}"].rearrange("(m p) j -> p m j", p=128),
                i8all[:])
    nc.compile()
    return nc


_NC_CACHE = {}


def _get_nc():
    if "nc" not in _NC_CACHE:
        _NC_CACHE["nc"] = _build_nc()
    return _NC_CACHE["nc"]


import ml_dtypes

BF16_NP = ml_dtypes.bfloat16


def _prep(lat, smp):
    """Build [e+1, NL] latent and [e+1, MS] sample operands (bf16)."""
    latK = np.concatenate(
        [lat.T, -0.5 * (lat.astype(np.float64) ** 2).sum(
            -1, keepdims=True).T.astype(np.float32)], axis=0)
    smpK = np.concatenate(
        [smp.T, np.ones((1, smp.shape[0]), np.float32)], axis=0)
    return (np.ascontiguousarray(latK.astype(BF16_NP)),
            np.ascontiguousarray(smpK.astype(BF16_NP)))


def _make_in_maps(latent_states, latent_actions, state_space_samples,
                  action_space_samples):
    in_maps = []
    for core in range(8):
        a, b = core % A_SHARDS, core // A_SHARDS
        latK_s, smpK_s = _prep(latent_states[a * NL:(a + 1) * NL],
                               state_space_samples[b * MS:(b + 1) * MS])
        latK_a, smpK_a = _prep(latent_actions[a * NL:(a + 1) * NL],
                               action_space_samples[b * MS:(b + 1) * MS])
        in_maps.append({"latK_s": latK_s, "smpK_s": smpK_s,
                        "latK_a": latK_a, "smpK_a": smpK_a})
    return in_maps


def _size_loss(lat):
    norms = np.abs(lat.astype(np.float64)).sum(-1)
    viol = np.maximum(norms - 1.0, 0.0)
    return (viol ** 2).mean()


def _cov_host(results, lat_full, smp_full, key_v, key_i):
    """Merge per-core group candidates -> exact L1 -> coverage loss term."""
    S = NSMP
    vals = np.empty((S, A_SHARDS * 8), np.float32)
    gids = np.empty((S, A_SHARDS * 8), np.int64)
    for core in range(8):
        a, b = core % A_SHARDS, core // A_SHARDS
        vals[b * MS:(b + 1) * MS, a * 8:(a + 1) * 8] = \
            results[core][key_v].astype(np.float32)
        gids[b * MS:(b + 1) * MS, a * 8:(a + 1) * 8] = \
            results[core][key_i].astype(np.int64) + a * NL
    sel = np.argsort(-vals, axis=1)[:, :TOP_T]
    gsel = np.take_along_axis(gids, sel, axis=1)            # [S, T]
    # group g of shard a (base latent a*NL+g) holds latents {base + NGRP*j}
    cand = (gsel[:, :, None] +
            (NGRP * np.arange(G))[None, None, :]).reshape(S, TOP_T * G)
    tails = np.empty(S)
    sm4s = np.empty((S, 4))
    lat32 = np.ascontiguousarray(lat_full, np.float32)
    smp32 = np.ascontiguousarray(smp_full, np.float32)
    for i in range(0, S, 512):
        gl = lat32[cand[i:i + 512]]                         # [c, T*G, e]
        d = np.abs(smp32[i:i + 512, None, :] - gl).sum(-1, dtype=np.float64)
        part = np.partition(d, 3, axis=1)[:, :4]
        sm4s[i:i + 512] = np.sort(part, axis=1)
        tails[i:i + 512] = part.mean(-1)
    far = np.argsort(-tails)[:64]
    return (sm4s[far] ** 2).mean()


def kernel(latent_states, latent_actions, state_space_samples,
           action_space_samples, _want_results=False, _trace=False,
           _tmpdir=None):
    nc = _get_nc()
    in_maps = _make_in_maps(latent_states, latent_actions,
                            state_space_samples, action_space_samples)
    res = run_bass_kernel_spmd(nc, in_maps, core_ids=list(range(8)),
                               trace=_trace, tmpdir=_tmpdir)
    total = np.float64(0)
    total += _size_loss(latent_states) + _size_loss(latent_actions)
    total += _cov_host(res.results, latent_states, state_space_samples,
                       "gv_s", "gi_s")
    total += _cov_host(res.results, latent_actions, action_space_samples,
                       "gv_a", "gi_a")
    out = np.float32(total)
    if _want_results:
        return out, res
    return out
